# revision 17
# baseline (speedup 1.0000x reference)
"""Trainium2 Bass kernel for nn_DifferentiableAmbiguity.

Computes, for each batch b of a complex signal s = s_real + i*s_imag [B=16, N=1024]:
    chi[b,k,f] = |FFT_m(s[m] * conj(s[m-k mod N]))|^2,  fftshifted on (k,f),
                 normalized by its global max.

Algorithm (per batch, on one NeuronCore; batches sharded 2-per-core over 8 cores):
  X = W @ G, where W[k,m] = conj(s)[(m-k) mod N] (circulant) and
  G[m,f] = s[m] * exp(-2i pi m f / N) (chirp-modulated DFT columns).
  - G is built on-chip from host cos/sin tables via ScalarE/VectorE.
  - W stationary tiles are slices of sliding-window "Hankel" tiles
    T[p,j] = s_doubled[p+j]: every access pattern stays ascending (PE
    requires non-negative AP steps). Each 128-row k-block comes out with k
    reversed; a reversed-identity matmul (J-flip) restores order.
  - Hermitian symmetry chi[(-k)%N, (-f)%N] == chi[k,f]: compute only
    k in [0,640) (5 of 8 blocks), mirror the rest.
  - fftshift on f is baked into the tables; fftshift on k into the output
    DMA row addressing.
  - |X|^2 via ScalarE Square + VectorE add; global max via row-reduce +
    DMA partition transpose; normalization fused into writeout scaling.
  - The two batches are software-pipelined: batch-0's writeout (which waits
    on its global-max chain) is emitted in the middle of batch-1's k-blocks
    so the PE never stalls on it.
All matmuls in float32r (tf32-like): ~1.5e-4 matmul rel err, 2 cycles/row.
"""

import numpy as np

import bass_rust
import concourse.bass as bass
import concourse.mybir as mybir
import concourse.tile as tile
import concourse.bass_utils as bass_utils

B, N = 16, 1024
NCORES = 8
BPC = B // NCORES  # batches per core
C_OFFS = [0, 128, 256, 384, 512]  # computed k-blocks (k in [0, 640))
DS_LEN = 2176  # doubled-signal length: >= 128 + 2048 - 1

f32 = mybir.dt.float32
f32r = mybir.dt.float32r
ALU = mybir.AluOpType


def _split_excess_waits(nc):
    """Legalize for this walrus build: at most 1 sem-wait per instruction,
    0 on Drains; excess waits move to preceding NOPs on the same engine."""
    for f in nc.m.functions:
        for blk in f.blocks:
            insts = list(blk.instructions)
            new_insts = []
            changed = False
            for inst in insts:
                si = inst.sync_info
                waits = list(si.on_wait) if (si is not None and si.on_wait) else []
                keep_n = 0 if isinstance(inst, mybir.InstDrain) else 1
                if len(waits) > keep_n:
                    changed = True
                    extra = waits[: len(waits) - keep_n]
                    keep = waits[len(waits) - keep_n:]
                    for w in extra:
                        nop = mybir.InstNoOp(
                            name=nc.get_next_instruction_name(), ins=[], outs=[]
                        )
                        nop.engine = inst.engine
                        nop.sync_info = bass_rust.SyncInfo(on_wait=[w], on_update=[])
                        new_insts.append(nop)
                    inst.sync_info = bass_rust.SyncInfo(
                        on_wait=keep,
                        on_update=list(si.on_update) if si.on_update else [],
                    )
                new_insts.append(inst)
            if changed:
                blk.instructions = new_insts
    return nc


def build_nc():
    nc = bass.Bass("TRN2", target_bir_lowering=False, debug=False)

    dsr = nc.dram_tensor("dsr", [BPC, DS_LEN], f32r, kind="ExternalInput")
    dsi = nc.dram_tensor("dsi", [BPC, DS_LEN], f32r, kind="ExternalInput")
    dsni = nc.dram_tensor("dsni", [BPC, DS_LEN], f32r, kind="ExternalInput")
    scols = nc.dram_tensor("scols", [BPC, 128, 16], f32, kind="ExternalInput")
    ctab = nc.dram_tensor("ctab", [N, N], f32, kind="ExternalInput")
    stab = nc.dram_tensor("stab", [N, N], f32, kind="ExternalInput")
    jmat = nc.dram_tensor("jmat", [128, 128], f32r, kind="ExternalInput")
    out = nc.dram_tensor("out", [BPC, N, N], f32, kind="ExternalOutput")

    st_state = {}

    with tile.TileContext(nc) as tc:
        with (
            tc.tile_pool(name="const", bufs=1) as constp,
            tc.tile_pool(name="tp", bufs=1) as tp,
            tc.tile_pool(name="gp", bufs=1) as gp,
            tc.tile_pool(name="cs", bufs=2) as csp,
            tc.tile_pool(name="tmp", bufs=2) as tmpp,
            tc.tile_pool(name="chi", bufs=1) as chip,
            tc.tile_pool(name="sq", bufs=2) as sqp,
            tc.tile_pool(name="st", bufs=2) as stp,
            tc.tile_pool(name="ms", bufs=1) as msp,
            tc.tile_pool(name="sm", bufs=1) as smp,
            tc.tile_pool(name="ps", bufs=2, space="PSUM") as psp,
        ):
            tJ = constp.tile([128, 128], f32r, tag="jmat")
            nc.scalar.dma_start(tJ[:], jmat[:])

            def emit_load_and_g(b):
                s = {}
                # scol first: the G build (DVE) gates on it
                scol = smp.tile([128, 16], f32, tag=f"scol{b}")
                nc.sync.dma_start(scol[:], scols[b])
                # sliding-window Hankel tiles: T[p, j] = ds[b, p+j]
                Tsr = tp.tile([128, 2048], f32r, tag=f"tsr{b % 2}")
                Tsi = tp.tile([128, 2048], f32r, tag=f"tsi{b % 2}")
                Tnsi = tp.tile([128, 2048], f32r, tag=f"tnsi{b % 2}")
                nc.gpsimd.dma_start(Tsr[:], bass.AP(dsr, b * DS_LEN, [[1, 128], [1, 2048]]))
                nc.gpsimd.dma_start(Tsi[:], bass.AP(dsi, b * DS_LEN, [[1, 128], [1, 2048]]))
                nc.gpsimd.dma_start(Tnsi[:], bass.AP(dsni, b * DS_LEN, [[1, 128], [1, 2048]]))
                s["T"] = (Tsr, Tsi, Tnsi)
                runmax = smp.tile([128, 1], f32, tag=f"runmax{b}")
                nc.vector.memset(runmax[:], 0.0)
                s["runmax"] = runmax

                s["G"] = [None] * 8
                s["chis"] = []
                s["scol"] = scol
                s["b"] = b
                return s

            def emit_g(s, mcs):
                # G[m,f] = s[m] * (C - iS)[m,f]
                b = s["b"]
                scol = s["scol"]
                for mc in mcs:
                    ct = csp.tile([128, N], f32, tag="ct")
                    st = csp.tile([128, N], f32, tag="st")
                    nc.sync.dma_start(ct[:], ctab[mc * 128:(mc + 1) * 128, :])
                    nc.scalar.dma_start(st[:], stab[mc * 128:(mc + 1) * 128, :])
                    sr_col = scol[:, mc:mc + 1]
                    si_col = scol[:, 8 + mc:9 + mc]
                    t1 = tmpp.tile([128, N], f32, tag="tt")
                    t2 = tmpp.tile([128, N], f32, tag="tt")
                    # G[0..1] are double-buffered across batches so the next
                    # batch's first chunks build during this batch's last block
                    gtag = f"{mc}"
                    gr = gp.tile([128, N], f32r, tag=f"gr{gtag}")
                    gi = gp.tile([128, N], f32r, tag=f"gi{gtag}")
                    nc.vector.tensor_scalar_mul(t1[:], st[:], si_col)  # t1 = S*si
                    nc.vector.scalar_tensor_tensor(     # Gr = C*sr + t1
                        gr[:], ct[:], sr_col, t1[:], op0=ALU.mult, op1=ALU.add
                    )
                    nc.vector.tensor_scalar_mul(t2[:], st[:], sr_col)  # t2 = S*sr
                    nc.vector.scalar_tensor_tensor(     # Gi = C*si - t2
                        gi[:], ct[:], si_col, t2[:], op0=ALU.mult, op1=ALU.subtract
                    )
                    s["G"][mc] = (gr, gi)

            def emit_kblock(b, s, kb):
                c = C_OFFS[kb]
                Tsr, Tsi, Tnsi = s["T"]
                xr0 = psp.tile([128, 512], f32, tag="xr0")
                xr1 = psp.tile([128, 512], f32, tag="xr1")
                xi0 = psp.tile([128, 512], f32, tag="xi0")
                xi1 = psp.tile([128, 512], f32, tag="xi1")
                for mc in range(8):
                    jb = 897 + mc * 128 - c
                    w_sr = Tsr[:, jb:jb + 128]
                    w_si = Tsi[:, jb:jb + 128]
                    w_nsi = Tnsi[:, jb:jb + 128]
                    gr, gi = s["G"][mc]
                    first = mc == 0
                    last = mc == 7
                    # Xr += Wr@Gr + (si)@Gi ; Xi += Wr@Gi + (-si)@Gr
                    nc.tensor.matmul(xr0[:], w_sr, gr[:, 0:512], start=first, stop=False)
                    nc.tensor.matmul(xr1[:], w_sr, gr[:, 512:1024], start=first, stop=False)
                    nc.tensor.matmul(xi0[:], w_sr, gi[:, 0:512], start=first, stop=False)
                    nc.tensor.matmul(xi1[:], w_sr, gi[:, 512:1024], start=first, stop=False)
                    nc.tensor.matmul(xr0[:], w_si, gi[:, 0:512], start=False, stop=last)
                    nc.tensor.matmul(xr1[:], w_si, gi[:, 512:1024], start=False, stop=last)
                    nc.tensor.matmul(xi0[:], w_nsi, gr[:, 0:512], start=False, stop=last)
                    nc.tensor.matmul(xi1[:], w_nsi, gr[:, 512:1024], start=False, stop=last)

                # epilogue: chi = Xr^2 + Xi^2 (k descending), running max
                chi_t = chip.tile([128, N], f32r, tag=f"chi{(5 * b + kb) % 8}")
                runmax = s["runmax"]
                for h, (xr, xi) in enumerate(((xr0, xi0), (xr1, xi1))):
                    hs = 512 * h
                    sqr = sqp.tile([128, 512], f32, tag="sqr")
                    sqi = sqp.tile([128, 512], f32, tag="sqi")
                    nc.scalar.square(sqr[:], xr[:])
                    nc.scalar.square(sqi[:], xi[:])
                    nc.vector.tensor_add(chi_t[:, hs:hs + 512], sqr[:], sqi[:])
                    tmax = smp.tile([128, 1], f32, tag=f"tmax{b}")
                    nc.vector.tensor_reduce(
                        tmax[:], chi_t[:, hs:hs + 512],
                        axis=mybir.AxisListType.X, op=ALU.max,
                    )
                    nc.vector.tensor_max(runmax[:], runmax[:], tmax[:])
                s["chis"].append(chi_t)

            def emit_finalize(b, s):
                # global max -> 1/max broadcast to all partitions
                runmax = s["runmax"]
                row = smp.tile([1, 128], f32, tag=f"rowmax{b}")
                nc.sync.dma_start(row[:], runmax[:])
                gmax = smp.tile([1, 1], f32, tag=f"gmax{b}")
                nc.vector.tensor_reduce(
                    gmax[:], row[:], axis=mybir.AxisListType.X, op=ALU.max
                )
                bmax = smp.tile([128, 1], f32, tag=f"bmax{b}")
                nc.sync.dma_start(
                    bmax[:], bass.AP(gmax[:].tensor, gmax[:].offset, [[1, 1], [0, 128]])
                )
                binv = smp.tile([128, 1], f32, tag=f"binv{b}")
                nc.vector.reciprocal(binv[:], bmax[:])
                s["binv"] = binv

            def emit_direct(b, s, kbs):
                binv = s["binv"]
                chis = s["chis"]
                # direct rows: J-flip to ascending k, scale (ScalarE), store
                for kb in kbs:
                    c = C_OFFS[kb]
                    stg = stp.tile([128, N], f32, tag="stg")
                    for h in range(2):
                        hs = 512 * h
                        jy = psp.tile([128, 512], f32, tag=("xr0" if h == 0 else "xr1"))
                        nc.tensor.matmul(
                            jy[:], tJ[:], chis[kb][:, hs:hs + 512], start=True, stop=True
                        )
                        if h == 0:
                            nc.scalar.mul(stg[:, hs:hs + 512], jy[:], binv[:])
                        else:
                            nc.vector.tensor_scalar_mul(stg[:, hs:hs + 512], jy[:], binv[:])
                    r0 = (c + 512) % N
                    eng = nc.sync if kb % 2 == 0 else nc.scalar
                    eng.dma_start(out[b, r0:r0 + 128, :], stg[:])

            def emit_mirror_flip(b, s):
                # mirror rows: f-reverse chi[k2] (no scale yet; binv not needed)
                s["ms"] = []
                for kb, c in enumerate(C_OFFS[:4]):
                    chi_t = s["chis"][kb]
                    ms = msp.tile([128, N], f32, tag=f"ms{kb}")
                    ap = chi_t[:]
                    rev = bass.AP(ap.tensor, ap.offset + 1023, [ap.ap[0], [-1, 1023]])
                    nc.vector.tensor_copy(ms[:, 0:1], chi_t[:, 0:1])
                    nc.vector.tensor_copy(ms[:, 1:1024], rev)
                    s["ms"].append(ms)

            def emit_mirror_store(b, s):
                binv = s["binv"]
                # scale in place (ScalarE, plain strides), then store;
                # dest row = 385 - c + r  (r = source partition)
                for kb, c in enumerate(C_OFFS[:4]):
                    ms = s["ms"][kb]
                    nc.scalar.mul(ms[:], ms[:], binv[:])
                    eng = nc.sync if kb % 2 == 1 else nc.scalar
                    if c == 0:
                        eng.dma_start(out[b, 385:512, :], ms[0:127, :])
                    elif c == 384:
                        eng.dma_start(out[b, 128:129, :], ms[127:128, :])
                    else:
                        r0 = 385 - c
                        eng.dma_start(out[b, r0:r0 + 128, :], ms[:])

            # --- software-pipelined schedule over the two batches:
            # batch-0's writeout is spread between batch-1's k-blocks so the
            # PE never waits on the normalization chain or PSUM slot reuse;
            # batch-1's first G chunks build during batch-0's last k-block.
            s0 = emit_load_and_g(0)
            emit_g(s0, range(8))
            emit_kblock(0, s0, 0)
            emit_kblock(0, s0, 1)
            s1 = emit_load_and_g(1)
            for kb in range(2, 5):
                emit_kblock(0, s0, kb)
            emit_finalize(0, s0)
            emit_mirror_flip(0, s0)
            emit_g(s1, range(8))
            emit_kblock(1, s1, 0)
            emit_kblock(1, s1, 1)
            emit_direct(0, s0, [0, 1])
            emit_kblock(1, s1, 2)
            emit_direct(0, s0, [2, 3])
            emit_kblock(1, s1, 3)
            emit_direct(0, s0, [4])
            emit_mirror_store(0, s0)
            emit_mirror_flip(1, s1)
            emit_kblock(1, s1, 4)
            emit_finalize(1, s1)
            emit_direct(1, s1, [0, 1, 2, 3, 4])
            emit_mirror_store(1, s1)

    _split_excess_waits(nc)
    return nc


_NC_CACHE = {}


def _get_nc():
    if "nc" not in _NC_CACHE:
        _NC_CACHE["nc"] = build_nc()
    return _NC_CACHE["nc"]


def _get_tables():
    if "tabs" not in _NC_CACHE:
        m = np.arange(N)[:, None].astype(np.float64)
        fo = np.arange(N)[None, :].astype(np.float64)
        fsh = (fo + N // 2) % N
        ang = 2.0 * np.pi * ((m * fsh) % N) / N
        _NC_CACHE["tabs"] = (
            np.cos(ang).astype(np.float32),
            np.sin(ang).astype(np.float32),
            np.eye(128, dtype=np.float32)[::-1].copy(),
        )
    return _NC_CACHE["tabs"]


def kernel(s_real: np.ndarray, s_imag: np.ndarray) -> np.ndarray:
    s_real = np.asarray(s_real, dtype=np.float32)
    s_imag = np.asarray(s_imag, dtype=np.float32)
    ctab, stab, jnp_ = _get_tables()
    nc = _get_nc()

    in_maps = []
    for core in range(NCORES):
        sl = slice(core * BPC, (core + 1) * BPC)
        sr = s_real[sl]  # [BPC, N]
        si = s_imag[sl]
        dsr = np.tile(sr, (1, 3))[:, :DS_LEN].copy()
        dsi_ = np.tile(si, (1, 3))[:, :DS_LEN].copy()
        dsni = -dsi_
        # scols[b, p, mc] = sr[b, mc*128+p]; cols 8..15 the same for si
        scols = np.concatenate(
            [
                sr.reshape(BPC, 8, 128).transpose(0, 2, 1),
                si.reshape(BPC, 8, 128).transpose(0, 2, 1),
            ],
            axis=2,
        ).copy()
        in_maps.append(
            {
                "dsr": dsr,
                "dsi": dsi_,
                "dsni": dsni,
                "scols": scols.astype(np.float32),
                "ctab": ctab,
                "stab": stab,
                "jmat": jnp_,
            }
        )

    res = bass_utils.run_bass_kernel_spmd(nc, in_maps, core_ids=list(range(NCORES)))
    return np.concatenate([r["out"] for r in res.results], axis=0)


# revision 19
# speedup vs baseline: 1.0105x; 1.0105x over previous
"""Trainium2 Bass kernel for nn_DifferentiableAmbiguity.

Computes, for each batch b of a complex signal s = s_real + i*s_imag [B=16, N=1024]:
    chi[b,k,f] = |FFT_m(s[m] * conj(s[m-k mod N]))|^2,  fftshifted on (k,f),
                 normalized by its global max.

Algorithm (per batch, on one NeuronCore; batches sharded 2-per-core over 8 cores):
  X = W @ G, where W[k,m] = conj(s)[(m-k) mod N] (circulant) and
  G[m,f] = s[m] * exp(-2i pi m f / N) (chirp-modulated DFT columns).
  - G is built on-chip from host cos/sin tables via ScalarE/VectorE.
  - W stationary tiles are slices of sliding-window "Hankel" tiles
    T[p,j] = s_doubled[p+j]: every access pattern stays ascending (PE
    requires non-negative AP steps). Each 128-row k-block comes out with k
    reversed; a reversed-identity matmul (J-flip) restores order.
  - Hermitian symmetry chi[(-k)%N, (-f)%N] == chi[k,f]: compute only
    k in [0,640) (5 of 8 blocks), mirror the rest.
  - fftshift on f is baked into the tables; fftshift on k into the output
    DMA row addressing.
  - |X|^2 via ScalarE Square + VectorE add; global max via row-reduce +
    DMA partition transpose; normalization fused into writeout scaling.
  - The two batches are software-pipelined: batch-0's writeout (which waits
    on its global-max chain) is emitted in the middle of batch-1's k-blocks
    so the PE never stalls on it.
All matmuls in float32r (tf32-like): ~1.5e-4 matmul rel err, 2 cycles/row.
"""

import numpy as np

import bass_rust
import concourse.bass as bass
import concourse.mybir as mybir
import concourse.tile as tile
import concourse.bass_utils as bass_utils

B, N = 16, 1024
NCORES = 8
BPC = B // NCORES  # batches per core
C_OFFS = [0, 128, 256, 384, 512]  # computed k-blocks (k in [0, 640))
DS_LEN = 2176  # doubled-signal length: >= 128 + 2048 - 1

f32 = mybir.dt.float32
f32r = mybir.dt.float32r
ALU = mybir.AluOpType


def _split_excess_waits(nc):
    """Legalize for this walrus build: at most 1 sem-wait per instruction,
    0 on Drains; excess waits move to preceding NOPs on the same engine."""
    for f in nc.m.functions:
        for blk in f.blocks:
            insts = list(blk.instructions)
            new_insts = []
            changed = False
            for inst in insts:
                si = inst.sync_info
                waits = list(si.on_wait) if (si is not None and si.on_wait) else []
                keep_n = 0 if isinstance(inst, mybir.InstDrain) else 1
                if len(waits) > keep_n:
                    changed = True
                    extra = waits[: len(waits) - keep_n]
                    keep = waits[len(waits) - keep_n:]
                    for w in extra:
                        nop = mybir.InstNoOp(
                            name=nc.get_next_instruction_name(), ins=[], outs=[]
                        )
                        nop.engine = inst.engine
                        nop.sync_info = bass_rust.SyncInfo(on_wait=[w], on_update=[])
                        new_insts.append(nop)
                    inst.sync_info = bass_rust.SyncInfo(
                        on_wait=keep,
                        on_update=list(si.on_update) if si.on_update else [],
                    )
                new_insts.append(inst)
            if changed:
                blk.instructions = new_insts
    return nc


def build_nc():
    nc = bass.Bass("TRN2", target_bir_lowering=False, debug=False)

    dsr = nc.dram_tensor("dsr", [BPC, DS_LEN], f32r, kind="ExternalInput")
    dsi = nc.dram_tensor("dsi", [BPC, DS_LEN], f32r, kind="ExternalInput")
    dsni = nc.dram_tensor("dsni", [BPC, DS_LEN], f32r, kind="ExternalInput")
    scols = nc.dram_tensor("scols", [BPC, 128, 16], f32, kind="ExternalInput")
    ctab = nc.dram_tensor("ctab", [N, N], f32, kind="ExternalInput")
    stab = nc.dram_tensor("stab", [N, N], f32, kind="ExternalInput")
    jmat = nc.dram_tensor("jmat", [128, 128], f32r, kind="ExternalInput")
    out = nc.dram_tensor("out", [BPC, N, N], f32, kind="ExternalOutput")

    st_state = {}

    with tile.TileContext(nc) as tc:
        with (
            tc.tile_pool(name="const", bufs=1) as constp,
            tc.tile_pool(name="tp", bufs=1) as tp,
            tc.tile_pool(name="gp", bufs=1) as gp,
            tc.tile_pool(name="cs", bufs=2) as csp,
            tc.tile_pool(name="tmp", bufs=2) as tmpp,
            tc.tile_pool(name="chi", bufs=1) as chip,
            tc.tile_pool(name="sq", bufs=1) as sqp,
            tc.tile_pool(name="st", bufs=2) as stp,
            tc.tile_pool(name="ms", bufs=1) as msp,
            tc.tile_pool(name="sm", bufs=1) as smp,
            tc.tile_pool(name="ps", bufs=2, space="PSUM") as psp,
        ):
            tJ = constp.tile([128, 128], f32r, tag="jmat")
            nc.scalar.dma_start(tJ[:], jmat[:])

            def emit_load_and_g(b):
                s = {}
                # scol first: the G build (DVE) gates on it
                scol = smp.tile([128, 16], f32, tag=f"scol{b}")
                nc.sync.dma_start(scol[:], scols[b])
                # sliding-window Hankel tiles: T[p, j] = ds[b, p+j]
                Tsr = tp.tile([128, 2048], f32r, tag=f"tsr{b % 2}")
                Tsi = tp.tile([128, 2048], f32r, tag=f"tsi{b % 2}")
                Tnsi = tp.tile([128, 2048], f32r, tag=f"tnsi{b % 2}")
                nc.gpsimd.dma_start(Tsr[:], bass.AP(dsr, b * DS_LEN, [[1, 128], [1, 2048]]))
                nc.gpsimd.dma_start(Tsi[:], bass.AP(dsi, b * DS_LEN, [[1, 128], [1, 2048]]))
                nc.gpsimd.dma_start(Tnsi[:], bass.AP(dsni, b * DS_LEN, [[1, 128], [1, 2048]]))
                s["T"] = (Tsr, Tsi, Tnsi)
                runmax = smp.tile([128, 1], f32, tag=f"runmax{b}")
                nc.vector.memset(runmax[:], 0.0)
                s["runmax"] = runmax

                s["G"] = [None] * 8
                s["chis"] = []
                s["scol"] = scol
                s["b"] = b
                return s

            def emit_g(s, mcs):
                # G[m,f] = s[m] * (C - iS)[m,f]
                b = s["b"]
                scol = s["scol"]
                for mc in mcs:
                    ct = csp.tile([128, N], f32, tag="ct")
                    st = csp.tile([128, N], f32, tag="st")
                    nc.sync.dma_start(ct[:], ctab[mc * 128:(mc + 1) * 128, :])
                    nc.scalar.dma_start(st[:], stab[mc * 128:(mc + 1) * 128, :])
                    sr_col = scol[:, mc:mc + 1]
                    si_col = scol[:, 8 + mc:9 + mc]
                    t1 = tmpp.tile([128, N], f32, tag="tt")
                    t2 = tmpp.tile([128, N], f32, tag="tt")
                    # G[0..1] are double-buffered across batches so the next
                    # batch's first chunks build during this batch's last block
                    gtag = f"{b % 2}_{mc}" if mc < 2 else f"{mc}"
                    gr = gp.tile([128, N], f32r, tag=f"gr{gtag}")
                    gi = gp.tile([128, N], f32r, tag=f"gi{gtag}")
                    nc.vector.tensor_scalar_mul(t1[:], st[:], si_col)  # t1 = S*si
                    nc.vector.scalar_tensor_tensor(     # Gr = C*sr + t1
                        gr[:], ct[:], sr_col, t1[:], op0=ALU.mult, op1=ALU.add
                    )
                    nc.vector.tensor_scalar_mul(t2[:], st[:], sr_col)  # t2 = S*sr
                    nc.vector.scalar_tensor_tensor(     # Gi = C*si - t2
                        gi[:], ct[:], si_col, t2[:], op0=ALU.mult, op1=ALU.subtract
                    )
                    s["G"][mc] = (gr, gi)

            def emit_kblock(b, s, kb):
                c = C_OFFS[kb]
                Tsr, Tsi, Tnsi = s["T"]
                xr0 = psp.tile([128, 512], f32, tag="xr0")
                xr1 = psp.tile([128, 512], f32, tag="xr1")
                xi0 = psp.tile([128, 512], f32, tag="xi0")
                xi1 = psp.tile([128, 512], f32, tag="xi1")
                for mc in range(8):
                    jb = 897 + mc * 128 - c
                    w_sr = Tsr[:, jb:jb + 128]
                    w_si = Tsi[:, jb:jb + 128]
                    w_nsi = Tnsi[:, jb:jb + 128]
                    gr, gi = s["G"][mc]
                    first = mc == 0
                    last = mc == 7
                    # Xr += Wr@Gr + (si)@Gi ; Xi += Wr@Gi + (-si)@Gr
                    nc.tensor.matmul(xr0[:], w_sr, gr[:, 0:512], start=first, stop=False)
                    nc.tensor.matmul(xr1[:], w_sr, gr[:, 512:1024], start=first, stop=False)
                    nc.tensor.matmul(xi0[:], w_sr, gi[:, 0:512], start=first, stop=False)
                    nc.tensor.matmul(xi1[:], w_sr, gi[:, 512:1024], start=first, stop=False)
                    nc.tensor.matmul(xr0[:], w_si, gi[:, 0:512], start=False, stop=last)
                    nc.tensor.matmul(xr1[:], w_si, gi[:, 512:1024], start=False, stop=last)
                    nc.tensor.matmul(xi0[:], w_nsi, gr[:, 0:512], start=False, stop=last)
                    nc.tensor.matmul(xi1[:], w_nsi, gr[:, 512:1024], start=False, stop=last)

                # epilogue: chi = Xr^2 + Xi^2 (k descending), running max
                chi_t = chip.tile([128, N], f32r, tag=f"chi{(5 * b + kb) % 6}")
                runmax = s["runmax"]
                for h, (xr, xi) in enumerate(((xr0, xi0), (xr1, xi1))):
                    hs = 512 * h
                    sqr = sqp.tile([128, 512], f32, tag="sqr")
                    sqi = sqp.tile([128, 512], f32, tag="sqi")
                    nc.scalar.square(sqr[:], xr[:])
                    nc.scalar.square(sqi[:], xi[:])
                    nc.vector.tensor_add(chi_t[:, hs:hs + 512], sqr[:], sqi[:])
                    tmax = smp.tile([128, 1], f32, tag=f"tmax{b}")
                    nc.vector.tensor_reduce(
                        tmax[:], chi_t[:, hs:hs + 512],
                        axis=mybir.AxisListType.X, op=ALU.max,
                    )
                    nc.vector.tensor_max(runmax[:], runmax[:], tmax[:])
                s["chis"].append(chi_t)

            def emit_finalize(b, s):
                # global max -> 1/max broadcast to all partitions
                runmax = s["runmax"]
                row = smp.tile([1, 128], f32, tag=f"rowmax{b}")
                nc.sync.dma_start(row[:], runmax[:])
                gmax = smp.tile([1, 1], f32, tag=f"gmax{b}")
                nc.vector.tensor_reduce(
                    gmax[:], row[:], axis=mybir.AxisListType.X, op=ALU.max
                )
                bmax = smp.tile([128, 1], f32, tag=f"bmax{b}")
                nc.sync.dma_start(
                    bmax[:], bass.AP(gmax[:].tensor, gmax[:].offset, [[1, 1], [0, 128]])
                )
                binv = smp.tile([128, 1], f32, tag=f"binv{b}")
                nc.vector.reciprocal(binv[:], bmax[:])
                s["binv"] = binv

            def emit_direct(b, s, kbs):
                binv = s["binv"]
                chis = s["chis"]
                # direct rows: J-flip to ascending k, scale (ScalarE), store
                for kb in kbs:
                    c = C_OFFS[kb]
                    stg = stp.tile([128, N], f32, tag="stg")
                    for h in range(2):
                        hs = 512 * h
                        jy = psp.tile([128, 512], f32, tag=("xr0" if h == 0 else "xr1"))
                        nc.tensor.matmul(
                            jy[:], tJ[:], chis[kb][:, hs:hs + 512], start=True, stop=True
                        )
                        if h == 0:
                            nc.scalar.mul(stg[:, hs:hs + 512], jy[:], binv[:])
                        else:
                            nc.vector.tensor_scalar_mul(stg[:, hs:hs + 512], jy[:], binv[:])
                    r0 = (c + 512) % N
                    eng = nc.sync if kb % 2 == 0 else nc.scalar
                    eng.dma_start(out[b, r0:r0 + 128, :], stg[:])

            def emit_mirror_flip(b, s):
                # mirror rows: f-reverse chi[k2] (no scale yet; binv not needed)
                s["ms"] = []
                for kb, c in enumerate(C_OFFS[:4]):
                    chi_t = s["chis"][kb]
                    ms = msp.tile([128, N], f32, tag=f"ms{kb}")
                    ap = chi_t[:]
                    rev = bass.AP(ap.tensor, ap.offset + 1023, [ap.ap[0], [-1, 1023]])
                    nc.vector.tensor_copy(ms[:, 0:1], chi_t[:, 0:1])
                    nc.vector.tensor_copy(ms[:, 1:1024], rev)
                    s["ms"].append(ms)

            def emit_mirror_store(b, s):
                binv = s["binv"]
                # scale in place (ScalarE, plain strides), then store;
                # dest row = 385 - c + r  (r = source partition)
                for kb, c in enumerate(C_OFFS[:4]):
                    ms = s["ms"][kb]
                    nc.scalar.mul(ms[:], ms[:], binv[:])
                    eng = nc.sync if kb % 2 == 1 else nc.scalar
                    if c == 0:
                        eng.dma_start(out[b, 385:512, :], ms[0:127, :])
                    elif c == 384:
                        eng.dma_start(out[b, 128:129, :], ms[127:128, :])
                    else:
                        r0 = 385 - c
                        eng.dma_start(out[b, r0:r0 + 128, :], ms[:])

            # --- software-pipelined schedule over the two batches:
            # batch-0's writeout is spread between batch-1's k-blocks so the
            # PE never waits on the normalization chain or PSUM slot reuse;
            # batch-1's first G chunks build during batch-0's last k-block.
            s0 = emit_load_and_g(0)
            emit_g(s0, range(8))
            emit_kblock(0, s0, 0)
            emit_kblock(0, s0, 1)
            s1 = emit_load_and_g(1)
            for kb in range(2, 5):
                emit_kblock(0, s0, kb)
            emit_g(s1, [0, 1])
            emit_finalize(0, s0)
            emit_mirror_flip(0, s0)
            emit_g(s1, range(2, 8))
            emit_kblock(1, s1, 0)
            emit_kblock(1, s1, 1)
            emit_direct(0, s0, [0, 1])
            emit_kblock(1, s1, 2)
            emit_direct(0, s0, [2, 3])
            emit_kblock(1, s1, 3)
            emit_direct(0, s0, [4])
            emit_mirror_store(0, s0)
            emit_mirror_flip(1, s1)
            emit_kblock(1, s1, 4)
            emit_finalize(1, s1)
            emit_direct(1, s1, [0, 1, 2, 3, 4])
            emit_mirror_store(1, s1)

    _split_excess_waits(nc)
    return nc


_NC_CACHE = {}


def _get_nc():
    if "nc" not in _NC_CACHE:
        _NC_CACHE["nc"] = build_nc()
    return _NC_CACHE["nc"]


def _get_tables():
    if "tabs" not in _NC_CACHE:
        m = np.arange(N)[:, None].astype(np.float64)
        fo = np.arange(N)[None, :].astype(np.float64)
        fsh = (fo + N // 2) % N
        ang = 2.0 * np.pi * ((m * fsh) % N) / N
        _NC_CACHE["tabs"] = (
            np.cos(ang).astype(np.float32),
            np.sin(ang).astype(np.float32),
            np.eye(128, dtype=np.float32)[::-1].copy(),
        )
    return _NC_CACHE["tabs"]


def kernel(s_real: np.ndarray, s_imag: np.ndarray) -> np.ndarray:
    s_real = np.asarray(s_real, dtype=np.float32)
    s_imag = np.asarray(s_imag, dtype=np.float32)
    ctab, stab, jnp_ = _get_tables()
    nc = _get_nc()

    in_maps = []
    for core in range(NCORES):
        sl = slice(core * BPC, (core + 1) * BPC)
        sr = s_real[sl]  # [BPC, N]
        si = s_imag[sl]
        dsr = np.tile(sr, (1, 3))[:, :DS_LEN].copy()
        dsi_ = np.tile(si, (1, 3))[:, :DS_LEN].copy()
        dsni = -dsi_
        # scols[b, p, mc] = sr[b, mc*128+p]; cols 8..15 the same for si
        scols = np.concatenate(
            [
                sr.reshape(BPC, 8, 128).transpose(0, 2, 1),
                si.reshape(BPC, 8, 128).transpose(0, 2, 1),
            ],
            axis=2,
        ).copy()
        in_maps.append(
            {
                "dsr": dsr,
                "dsi": dsi_,
                "dsni": dsni,
                "scols": scols.astype(np.float32),
                "ctab": ctab,
                "stab": stab,
                "jmat": jnp_,
            }
        )

    res = bass_utils.run_bass_kernel_spmd(nc, in_maps, core_ids=list(range(NCORES)))
    return np.concatenate([r["out"] for r in res.results], axis=0)
